# revision 14
# baseline (speedup 1.0000x reference)
"""Capsule-routing kernel for Trainium2, 8-way tensor-parallel over output capsules.

The reference's dynamic routing is inert: the logits `b` are only updated
*after* the final iteration's output is computed, so `b` stays zero and the
coupling coefficients are exactly uniform (1/J) in every iteration.  The whole
module therefore collapses to

    out[b, j, d] = squash_d( (1/J) * sum_{i,m} W[j, i, d, m] * x[b, i, m] )

i.e. one [B, I*M] @ [I*M, J*D] matmul followed by a per-(b, j) squash over D.

Sharding: the output-capsule axis J (32) is split 8 ways -> each core holds a
[I*M, 4*D] slice of W plus a replicated copy of x, computes its [B, 4, D]
output slice entirely locally (no collectives -- nothing couples the J shards
once the routing softmax is gone), and the host concatenates.

Datapath (v3):
 - fp16 inputs.  Halves the HBM stream vs fp32 AND runs the PE at 1 cycle/row
   instead of 4 (fp32 matmul = two half-rate passes).  Host casts; rel err
   ~4e-4 against the f32 reference (gate 2e-2).  W is pre-scaled by 1/J on
   the host so PSUM accumulates s directly and the squash needs no rescale.
 - x and W are interleaved per k-tile in ONE dram tensor zr[128, KT, 128]
   (first 64 columns = x k-tile [128, B], last 64 = W k-tile [128, JL*D]).
   One DMA descriptor then covers both tensors, per-partition contiguous
   runs are ch*256 B (>= 2 KB for ch >= 8), and the trigger count halves.
   The k-range is split between the two HWDGE rings (scalar ring: first
   chunks, sync ring: tail chunks) so both descriptor generators stream in
   parallel; the PE consumes k-tiles in order so the early-starting scalar
   ring feeds the first matmuls while the sync ring's tail chunks land.
 - PE accumulates all 72 k-tiles into one PSUM bank ([B=64, 64] f32); the
   squash reads PSUM directly (Square on ACT, final scale-multiply on DVE),
   with activation tables prefetched during the DMA stream and the
   reciprocal done by the fast custom-DVE approximation (no table).
 - Barrier-free epilogue on the SYNC engine (which also issues the output
   DMA): the four other engines bump a fin semaphore as proof they executed
   past their last wait, sync alone waits for the global tile clock + fin=4,
   clears the semaphores, and halts.
"""

import os
import numpy as np

B, I, M = 64, 1152, 8
J, D = 32, 16
NCORES = 8
JL = J // NCORES           # output capsules per core
K = I * M                  # contraction length 9216
KT = K // 128              # 72 k-tiles of 128

# experiment knobs (defaults = best configuration)
EPILOGUE = os.environ.get("CAPS_EPILOGUE", "finsem")  # stock | finsem | finsync
LEAN_INIT = os.environ.get("CAPS_LEANINIT", "1") == "1"  # skip init barrier
PREFETCH_TABLES = os.environ.get("CAPS_PREFETCH", "1") == "1"
RECIP = os.environ.get("CAPS_RECIP", "fast")          # fast | exact
# k-chunks per ring: scalar ring streams the head of the k range, sync ring
# the tail.  Bigger chunks -> bigger DMA packets (ch*256B per partition) and
# fewer ~600ns trigger instructions; the first chunk is smaller so the PE
# starts early.
_PLANS = {
    # (ring, n_ktiles) in k order; "s"=scalar HWDGE ring, "y"=sync ring.
    # Alternating rings keeps completion roughly in k order under engine
    # contention, so the PE is fed continuously instead of bursting after
    # the stream (a chunk only unblocks its matmuls once ALL its packets
    # landed, so chunk grain = PE stall grain).
    "i1": [("s", 8), ("y", 8), ("s", 12), ("y", 12), ("s", 12), ("y", 12),
           ("s", 8)],
    "i2": [("s", 8), ("y", 8), ("s", 8), ("y", 8), ("s", 8), ("y", 8),
           ("s", 8), ("y", 8), ("s", 8)],
    "p1": [("s", 8), ("s", 16), ("s", 16), ("y", 16), ("y", 16)],
}
PLAN = _PLANS[os.environ.get("CAPS_PLAN", "i1")]
assert sum(ch for _, ch in PLAN) == KT

_cache = {}


def _make_tile_context(nc):
    import concourse.tile as tile

    if EPILOGUE == "stock":
        return tile.TileContext(nc)

    final_eng = "gpsimd" if EPILOGUE == "finsem" else "sync"

    class FinTileContext(tile.TileContext):
        """Barrier-free tail.  Each other engine's final instruction
        increments a regular semaphore -- an increment is proof the engine
        executed past its last data-dependent wait.  The finalizer engine
        alone then waits for (a) the global clock (every tile semaphore at
        its final value, which covers all DMA completions including the
        output write) and (b) fin >= 4, clears the tile semaphores for
        re-execution, clears fin, and halts.  The other engines have
        already halted, so nothing can observe a cleared semaphore
        mid-wait.  finalizer=sync also owns the output DMA, so its global-
        clock wait is the natural last dependency of the program."""

        def _drain_and_barrier(self, tick_clock, wait_clock):
            from concourse.tile import ScopedClock
            from concourse.bass import compact_to_ranges

            nc = self.nc
            fin_eng = getattr(nc, final_eng)
            others = [e for e in (nc.sync, nc.tensor, nc.scalar, nc.vector,
                                  nc.gpsimd) if e is not fin_eng]
            fin = nc.alloc_semaphore("tile_fin")
            for eng in others:
                eng.nop().then_inc(fin, 1)
            drain_inst = fin_eng.drain()
            wait_clock.add_sem_waits(
                drain_inst.ins, ScopedClock({None: tick_clock.global_clock})
            )
            fin_eng.wait_ge(fin, 4)
            popped = nc._tile_sem_poison_stack.pop()
            assert popped is self._sem_poison
            sems = list(self.sems.allocated().values())
            sem_nums = [s.num if hasattr(s, "num") else s for s in sems]
            for sem_range in compact_to_ranges(sem_nums):
                assert nc._state.free_isdisjoint(sem_range)
                # dma_reset == drain(semaphore_range) but only the gpsimd
                # engine class exposes the alias
                fin_eng.drain(semaphore_range=sem_range)
                fin_eng.sem_clear(sem_range)
            nc._state.prepend_free_semaphores(sem_nums)
            for poison_set in nc._tile_sem_poison_stack:
                poison_set.update(sem_nums)
            fin_eng.sem_clear(fin)

    return FinTileContext(nc)


def _build_nc():
    import concourse.bacc as bacc
    from concourse import mybir

    f32 = mybir.dt.float32
    f16 = mybir.dt.float16
    if LEAN_INIT:
        # Bass.__init__ ends with const-AP memsets + an all-engine barrier
        # ordering them before use (~0.8us of head).  This kernel never
        # reads a const AP (all immediates are inline, Sqrt bias is an
        # explicit eps tile), so the barrier orders dead writes -- skip it.
        class LeanBacc(bacc.Bacc):
            _skip_init_barrier = False

            def all_engine_barrier(self, **kw):
                if LeanBacc._skip_init_barrier:
                    return
                super().all_engine_barrier(**kw)

        LeanBacc._skip_init_barrier = True
        try:
            nc = LeanBacc("TRN2", target_bir_lowering=False, debug=False,
                          num_devices=NCORES)
        finally:
            LeanBacc._skip_init_barrier = False
    else:
        nc = bacc.Bacc("TRN2", target_bir_lowering=False, debug=False,
                       num_devices=NCORES)
    zr = nc.dram_tensor("zr", [128, KT, 2 * B], f16, kind="ExternalInput").ap()
    out = nc.dram_tensor("out", [B, JL, D], f32, kind="ExternalOutput").ap()

    tc = _make_tile_context(nc)
    with tc:
        with tc.tile_pool(name="zin", bufs=len(PLAN)) as zpool, \
             tc.tile_pool(name="acc", bufs=1, space="PSUM") as ppool, \
             tc.tile_pool(name="sq", bufs=1) as spool:
            eps = spool.tile([B, 1], f32)
            nc.vector.memset(eps[:], 1e-7)

            psum = ppool.tile([B, JL, D], f32)

            plan = [(nc.scalar if r == "s" else nc.sync, ch)
                    for r, ch in PLAN]
            tiles = []
            k0 = 0
            for eng, ch in plan:
                zt = zpool.tile([128, ch, 2 * B], f16, tag="zt")
                eng.dma_start(out=zt[:], in_=zr[:, k0:k0 + ch, :])
                tiles.append((zt, ch))
                k0 += ch

            if PREFETCH_TABLES:
                # Prefetch the activation tables while DMAs stream (a table
                # load is ~1.3us and would otherwise land on the critical
                # tail).  Emitted AFTER the DMA issues so the table loads
                # don't delay the z stream on the scalar ring.
                dummy = spool.tile([B, 1], f32)
                nc.scalar.activation(dummy[:], eps[:],
                                     mybir.ActivationFunctionType.Sqrt,
                                     bias=eps[:])
                nc.scalar.activation(dummy[:], eps[:],
                                     mybir.ActivationFunctionType.Square,
                                     bias=eps[:])

            n = 0
            for zt, ch in tiles:
                for i in range(ch):
                    # psum[b, (j d)] += zt[k, i, 0:B].T @ zt[k, i, B:2B]
                    nc.tensor.matmul(psum[:], zt[:, i, 0:B], zt[:, i, B:2 * B],
                                     start=(n == 0), stop=(n == KT - 1))
                    n += 1

            # squash (W pre-scaled by 1/J on the host, so psum holds s):
            # norm = sum_d s^2;  out = s * norm / ((1+norm)*sqrt(norm+eps))
            sq = spool.tile([B, JL, D], f32)
            nc.scalar.activation(sq[:], psum[:],
                                 mybir.ActivationFunctionType.Square)
            norm = spool.tile([B, JL], f32)
            nc.vector.reduce_sum(norm[:], sq[:], axis=mybir.AxisListType.X)
            rt = spool.tile([B, JL], f32)
            nc.scalar.activation(rt[:], norm[:],
                                 mybir.ActivationFunctionType.Sqrt, bias=eps[:])
            np1 = spool.tile([B, JL], f32)
            nc.vector.tensor_scalar_add(np1[:], in0=norm[:], scalar1=1.0)
            den = spool.tile([B, JL], f32)
            nc.vector.tensor_mul(den[:], rt[:], np1[:])
            rden = spool.tile([B, JL], f32)
            if RECIP == "fast":
                # ~51 ULP is plenty (gate 2e-2); saves the iterative op
                nc.vector.reciprocal_approx_fast(out=rden[:], in_=den[:])
            else:
                nc.vector.reciprocal(rden[:], den[:])
            sc = spool.tile([B, JL], f32)
            nc.vector.tensor_mul(sc[:], norm[:], rden[:])
            o = spool.tile([B, JL, D], f32)
            nc.vector.tensor_mul(o[:], psum[:],
                                 sc[:].to_broadcast([B, JL, D]))
            # output write split across both HWDGE rings so the two halves'
            # trigger+flight latencies overlap
            nc.scalar.dma_start(out=out[0:B // 2], in_=o[0:B // 2])
            nc.sync.dma_start(out=out[B // 2:B], in_=o[B // 2:B])

    nc.compile()
    return nc


def _get_nc():
    if "nc" not in _cache:
        _cache["nc"] = _build_nc()
    return _cache["nc"]


def _ktile(a2d):
    # [K, F] -> [128, KT, F] so SBUF partition p of k-tile n holds row n*128+p
    f = a2d.shape[1]
    return np.ascontiguousarray(a2d.reshape(KT, 128, f).transpose(1, 0, 2))


def make_in_maps(x, W):
    x = np.asarray(x, dtype=np.float32)
    W = np.asarray(W, dtype=np.float32) * np.float32(1.0 / J)
    x2d = x.transpose(1, 2, 0).reshape(K, B)                 # k=(i,m) rows
    in_maps = []
    for c in range(NCORES):
        wc = W[c * JL:(c + 1) * JL]                          # [JL, I, D, M]
        w2d = wc.transpose(1, 3, 0, 2).reshape(K, JL * D)
        z = np.concatenate([x2d, w2d], axis=1).astype(np.float16)
        in_maps.append({"zr": _ktile(z)})
    return in_maps


def run_sharded(x, W, trace=False, **run_kwargs):
    from concourse.bass_utils import run_bass_kernel_spmd

    nc = _get_nc()
    res = run_bass_kernel_spmd(nc, make_in_maps(x, W),
                               list(range(NCORES)), trace=trace, **run_kwargs)
    outs = [np.asarray(r["out"], dtype=np.float32) for r in res.results]
    full = np.stack(outs, axis=1).reshape(B, J, D)
    return full, res


def kernel(**inputs):
    out, _ = run_sharded(inputs["x"], inputs["W"])
    return out


# revision 15
# speedup vs baseline: 1.0748x; 1.0748x over previous
"""Capsule-routing kernel for Trainium2, 8-way tensor-parallel over output capsules.

The reference's dynamic routing is inert: the logits `b` are only updated
*after* the final iteration's output is computed, so `b` stays zero and the
coupling coefficients are exactly uniform (1/J) in every iteration.  The whole
module therefore collapses to

    out[b, j, d] = squash_d( (1/J) * sum_{i,m} W[j, i, d, m] * x[b, i, m] )

i.e. one [B, I*M] @ [I*M, J*D] matmul followed by a per-(b, j) squash over D.

Sharding: the output-capsule axis J (32) is split 8 ways -> each core holds a
[I*M, 4*D] slice of W plus a replicated copy of x, computes its [B, 4, D]
output slice entirely locally (no collectives -- nothing couples the J shards
once the routing softmax is gone), and the host concatenates.

Datapath (v3):
 - fp16 inputs.  Halves the HBM stream vs fp32 AND runs the PE at 1 cycle/row
   instead of 4 (fp32 matmul = two half-rate passes).  Host casts; rel err
   ~4e-4 against the f32 reference (gate 2e-2).  W is pre-scaled by 1/J on
   the host so PSUM accumulates s directly and the squash needs no rescale.
 - x and W are interleaved per k-tile in ONE dram tensor zr[128, KT, 128]
   (first 64 columns = x k-tile [128, B], last 64 = W k-tile [128, JL*D]).
   One DMA descriptor then covers both tensors, per-partition contiguous
   runs are ch*256 B (>= 2 KB for ch >= 8), and the trigger count halves.
   The k-range is split between the two HWDGE rings (scalar ring: first
   chunks, sync ring: tail chunks) so both descriptor generators stream in
   parallel; the PE consumes k-tiles in order so the early-starting scalar
   ring feeds the first matmuls while the sync ring's tail chunks land.
 - PE accumulates all 72 k-tiles into one PSUM bank ([B=64, 64] f32); the
   squash reads PSUM directly (Square on ACT, final scale-multiply on DVE),
   with activation tables prefetched during the DMA stream and the
   reciprocal done by the fast custom-DVE approximation (no table).
 - Barrier-free epilogue on the SYNC engine (which also issues the output
   DMA): the four other engines bump a fin semaphore as proof they executed
   past their last wait, sync alone waits for the global tile clock + fin=4,
   clears the semaphores, and halts.
"""

import os
import numpy as np

B, I, M = 64, 1152, 8
J, D = 32, 16
NCORES = 8
JL = J // NCORES           # output capsules per core
K = I * M                  # contraction length 9216
KT = K // 128              # 72 k-tiles of 128

# experiment knobs (defaults = best configuration)
EPILOGUE = os.environ.get("CAPS_EPILOGUE", "finsem")  # stock | finsem | finsync
LEAN_INIT = os.environ.get("CAPS_LEANINIT", "1") == "1"  # skip init barrier
PREFETCH_TABLES = os.environ.get("CAPS_PREFETCH", "1") == "1"
RECIP = os.environ.get("CAPS_RECIP", "fast")          # fast | exact
# k-chunks per ring: scalar ring streams the head of the k range, sync ring
# the tail.  Bigger chunks -> bigger DMA packets (ch*256B per partition) and
# fewer ~600ns trigger instructions; the first chunk is smaller so the PE
# starts early.
_PLANS = {
    # (ring, n_ktiles) in k order; "s"=scalar HWDGE ring, "y"=sync ring.
    # Alternating rings keeps completion roughly in k order under engine
    # contention, so the PE is fed continuously instead of bursting after
    # the stream (a chunk only unblocks its matmuls once ALL its packets
    # landed, so chunk grain = PE stall grain).
    "i1": [("s", 8), ("y", 8), ("s", 12), ("y", 12), ("s", 12), ("y", 12),
           ("s", 8)],
    "i2": [("s", 8), ("y", 8), ("s", 8), ("y", 8), ("s", 8), ("y", 8),
           ("s", 8), ("y", 8), ("s", 8)],
    # tapered tail: the last chunks are small so the PE (and then the
    # squash) trail the final DMA byte by as little as possible
    "i3": [("s", 8), ("y", 8), ("s", 12), ("y", 12), ("s", 12), ("y", 8),
           ("s", 8), ("y", 4)],
    "p1": [("s", 8), ("s", 16), ("s", 16), ("y", 16), ("y", 16)],
}
PLAN = _PLANS[os.environ.get("CAPS_PLAN", "i3")]
assert sum(ch for _, ch in PLAN) == KT

_cache = {}


def _make_tile_context(nc):
    import concourse.tile as tile

    if EPILOGUE == "stock":
        return tile.TileContext(nc)

    final_eng = "gpsimd" if EPILOGUE == "finsem" else "sync"

    class FinTileContext(tile.TileContext):
        """Barrier-free tail.  Each other engine's final instruction
        increments a regular semaphore -- an increment is proof the engine
        executed past its last data-dependent wait.  The finalizer engine
        alone then waits for (a) the global clock (every tile semaphore at
        its final value, which covers all DMA completions including the
        output write) and (b) fin >= 4, clears the tile semaphores for
        re-execution, clears fin, and halts.  The other engines have
        already halted, so nothing can observe a cleared semaphore
        mid-wait.  finalizer=sync also owns the output DMA, so its global-
        clock wait is the natural last dependency of the program."""

        def _drain_and_barrier(self, tick_clock, wait_clock):
            from concourse.tile import ScopedClock
            from concourse.bass import compact_to_ranges

            nc = self.nc
            fin_eng = getattr(nc, final_eng)
            others = [e for e in (nc.sync, nc.tensor, nc.scalar, nc.vector,
                                  nc.gpsimd) if e is not fin_eng]
            fin = nc.alloc_semaphore("tile_fin")
            for eng in others:
                eng.nop().then_inc(fin, 1)
            drain_inst = fin_eng.drain()
            wait_clock.add_sem_waits(
                drain_inst.ins, ScopedClock({None: tick_clock.global_clock})
            )
            fin_eng.wait_ge(fin, 4)
            popped = nc._tile_sem_poison_stack.pop()
            assert popped is self._sem_poison
            sems = list(self.sems.allocated().values())
            sem_nums = [s.num if hasattr(s, "num") else s for s in sems]
            for sem_range in compact_to_ranges(sem_nums):
                assert nc._state.free_isdisjoint(sem_range)
                # dma_reset == drain(semaphore_range) but only the gpsimd
                # engine class exposes the alias
                fin_eng.drain(semaphore_range=sem_range)
                fin_eng.sem_clear(sem_range)
            nc._state.prepend_free_semaphores(sem_nums)
            for poison_set in nc._tile_sem_poison_stack:
                poison_set.update(sem_nums)
            fin_eng.sem_clear(fin)

    return FinTileContext(nc)


def _build_nc():
    import concourse.bacc as bacc
    from concourse import mybir

    f32 = mybir.dt.float32
    f16 = mybir.dt.float16
    if LEAN_INIT:
        # Bass.__init__ ends with const-AP memsets + an all-engine barrier
        # ordering them before use (~0.8us of head).  This kernel never
        # reads a const AP (all immediates are inline, Sqrt bias is an
        # explicit eps tile), so the barrier orders dead writes -- skip it.
        class LeanBacc(bacc.Bacc):
            _skip_init_barrier = False

            def all_engine_barrier(self, **kw):
                if LeanBacc._skip_init_barrier:
                    return
                super().all_engine_barrier(**kw)

        LeanBacc._skip_init_barrier = True
        try:
            nc = LeanBacc("TRN2", target_bir_lowering=False, debug=False,
                          num_devices=NCORES)
        finally:
            LeanBacc._skip_init_barrier = False
    else:
        nc = bacc.Bacc("TRN2", target_bir_lowering=False, debug=False,
                       num_devices=NCORES)
    zr = nc.dram_tensor("zr", [128, KT, 2 * B], f16, kind="ExternalInput").ap()
    out = nc.dram_tensor("out", [B, JL, D], f32, kind="ExternalOutput").ap()

    tc = _make_tile_context(nc)
    with tc:
        with tc.tile_pool(name="zin", bufs=len(PLAN)) as zpool, \
             tc.tile_pool(name="acc", bufs=1, space="PSUM") as ppool, \
             tc.tile_pool(name="sq", bufs=1) as spool:
            eps = spool.tile([B, 1], f32)
            nc.vector.memset(eps[:], 1e-7)

            psum = ppool.tile([B, JL, D], f32)

            plan = [(nc.scalar if r == "s" else nc.sync, ch)
                    for r, ch in PLAN]
            tiles = []
            k0 = 0
            for eng, ch in plan:
                zt = zpool.tile([128, ch, 2 * B], f16, tag="zt")
                eng.dma_start(out=zt[:], in_=zr[:, k0:k0 + ch, :])
                tiles.append((zt, ch))
                k0 += ch

            if PREFETCH_TABLES:
                # Prefetch the activation tables while DMAs stream (a table
                # load is ~1.3us and would otherwise land on the critical
                # tail).  Emitted AFTER the DMA issues so the table loads
                # don't delay the z stream on the scalar ring.
                dummy = spool.tile([B, 1], f32)
                nc.scalar.activation(dummy[:], eps[:],
                                     mybir.ActivationFunctionType.Sqrt,
                                     bias=eps[:])
                nc.scalar.activation(dummy[:], eps[:],
                                     mybir.ActivationFunctionType.Square,
                                     bias=eps[:])

            n = 0
            for zt, ch in tiles:
                for i in range(ch):
                    # psum[b, (j d)] += zt[k, i, 0:B].T @ zt[k, i, B:2B]
                    nc.tensor.matmul(psum[:], zt[:, i, 0:B], zt[:, i, B:2 * B],
                                     start=(n == 0), stop=(n == KT - 1))
                    n += 1

            # squash (W pre-scaled by 1/J on the host, so psum holds s):
            # norm = sum_d s^2;  out = s * norm / ((1+norm)*sqrt(norm+eps))
            sq = spool.tile([B, JL, D], f32)
            nc.scalar.activation(sq[:], psum[:],
                                 mybir.ActivationFunctionType.Square)
            norm = spool.tile([B, JL], f32)
            nc.vector.reduce_sum(norm[:], sq[:], axis=mybir.AxisListType.X)
            rt = spool.tile([B, JL], f32)
            nc.scalar.activation(rt[:], norm[:],
                                 mybir.ActivationFunctionType.Sqrt, bias=eps[:])
            np1 = spool.tile([B, JL], f32)
            nc.vector.tensor_scalar_add(np1[:], in0=norm[:], scalar1=1.0)
            den = spool.tile([B, JL], f32)
            nc.vector.tensor_mul(den[:], rt[:], np1[:])
            rden = spool.tile([B, JL], f32)
            if RECIP == "fast":
                # ~51 ULP is plenty (gate 2e-2); saves the iterative op
                nc.vector.reciprocal_approx_fast(out=rden[:], in_=den[:])
            else:
                nc.vector.reciprocal(rden[:], den[:])
            sc = spool.tile([B, JL], f32)
            nc.vector.tensor_mul(sc[:], norm[:], rden[:])
            o = spool.tile([B, JL, D], f32)
            nc.vector.tensor_mul(o[:], psum[:],
                                 sc[:].to_broadcast([B, JL, D]))
            # output write split across both HWDGE rings so the two halves'
            # trigger+flight latencies overlap
            nc.scalar.dma_start(out=out[0:B // 2], in_=o[0:B // 2])
            nc.sync.dma_start(out=out[B // 2:B], in_=o[B // 2:B])

    nc.compile()
    return nc


def _get_nc():
    if "nc" not in _cache:
        _cache["nc"] = _build_nc()
    return _cache["nc"]


def _ktile(a2d):
    # [K, F] -> [128, KT, F] so SBUF partition p of k-tile n holds row n*128+p
    f = a2d.shape[1]
    return np.ascontiguousarray(a2d.reshape(KT, 128, f).transpose(1, 0, 2))


def make_in_maps(x, W):
    x = np.asarray(x, dtype=np.float32)
    W = np.asarray(W, dtype=np.float32) * np.float32(1.0 / J)
    x2d = x.transpose(1, 2, 0).reshape(K, B)                 # k=(i,m) rows
    in_maps = []
    for c in range(NCORES):
        wc = W[c * JL:(c + 1) * JL]                          # [JL, I, D, M]
        w2d = wc.transpose(1, 3, 0, 2).reshape(K, JL * D)
        z = np.concatenate([x2d, w2d], axis=1).astype(np.float16)
        in_maps.append({"zr": _ktile(z)})
    return in_maps


def run_sharded(x, W, trace=False, **run_kwargs):
    from concourse.bass_utils import run_bass_kernel_spmd

    nc = _get_nc()
    res = run_bass_kernel_spmd(nc, make_in_maps(x, W),
                               list(range(NCORES)), trace=trace, **run_kwargs)
    outs = [np.asarray(r["out"], dtype=np.float32) for r in res.results]
    full = np.stack(outs, axis=1).reshape(B, J, D)
    return full, res


def kernel(**inputs):
    out, _ = run_sharded(inputs["x"], inputs["W"])
    return out
